# revision 8
# baseline (speedup 1.0000x reference)
"""MoE routing (capacity-drop dispatch/combine) kernel for 8 Trainium2 cores.

The reference module's expert compute is identity, so binned_gather followed by
binned_scatter algebraically reduces to a per-token scale:

    out[t] = (sum_k expert_weights[t,k] * within_capacity(t,k)) * x[t] + bias

within_capacity(t,k) is determined by the token's position in its expert's bin
under a stable sort of all (token, k) routing entries by expert id, i.e. by the
running per-expert count over the flat entry stream.  The kernel computes that
routing mask on-device with per-expert prefix scans (tensor_tensor_scan along
the free dim + a triangular-matmul carry across partitions), then streams x
through a fused (x * coeff + bias) elementwise pass.

Sharding: data-parallel over tokens; each of the 8 cores scales its own 2048
tokens.  The routing metadata (32K entries) is computed redundantly on every
core, so no collectives are needed.
"""

import numpy as np

import concourse.bass as bass
import concourse.bacc as bacc
import concourse.mybir as mybir
from concourse.tile import TileContext
from concourse.bass_utils import run_bass_kernel_spmd

AluOp = mybir.AluOpType
F32 = mybir.dt.float32
I32 = mybir.dt.int32

N_CORES = 8
B, N, D = 4, 4096, 1024
TOP_K = 2
E = 8
TOK = B * N                # 16384 tokens
T = TOK * TOP_K            # 32768 routing entries
CAP = T // E               # 4096 expert capacity
P = 128                    # partitions
CC = T // P                # 256 routing entries per partition row
TPC = TOK // N_CORES       # 2048 tokens per core
NT = TPC // P              # 16 x-tiles of [128, D] per core

_CACHE = {}


def _build_bass():
    nc = bacc.Bacc(None, target_bir_lowering=False)
    xs = nc.dram_tensor("xs", [TPC, D], F32, kind="ExternalInput")
    te = nc.dram_tensor("te", [P, CC], I32, kind="ExternalInput")
    wf = nc.dram_tensor("wf", [P, CC], F32, kind="ExternalInput")
    ut = nc.dram_tensor("ut", [P, P], F32, kind="ExternalInput")
    sel = nc.dram_tensor("sel", [P, NT], F32, kind="ExternalInput")
    bb = nc.dram_tensor("bb", [P, D], F32, kind="ExternalInput")
    ys = nc.dram_tensor("ys", [TPC, D], F32, kind="ExternalOutput")

    xt = xs.rearrange("(j p) d -> j p d", p=P)
    yt = ys.rearrange("(j p) d -> j p d", p=P)

    with TileContext(nc) as tc:
        with tc.tile_pool(name="const", bufs=1) as cpool, \
             tc.tile_pool(name="route", bufs=1) as rpool, \
             tc.tile_pool(name="acc", bufs=2) as apool, \
             tc.tile_pool(name="ps", bufs=1, space="PSUM") as ppool, \
             tc.tile_pool(name="xw", bufs=NT + 2) as xpool:
            te_sb = cpool.tile([P, CC], I32)
            nc.sync.dma_start(te_sb[:], te[:])
            w_sb = cpool.tile([P, CC], F32)
            nc.sync.dma_start(w_sb[:], wf[:])
            u_sb = cpool.tile([P, P], F32)
            nc.sync.dma_start(u_sb[:], ut[:])
            sel_sb = cpool.tile([P, NT], F32)
            nc.sync.dma_start(sel_sb[:], sel[:])
            b_sb = cpool.tile([P, D], F32)
            nc.sync.dma_start(b_sb[:], bb[:])

            # ---- routing: global capacity mask (redundant on every core) ----
            # Flat entry i = p*CC + c lives at [p, c]; stable-sort bin position
            # equals the global running count of entry's expert over i.
            zero = rpool.tile([P, CC], F32)
            nc.vector.memset(zero[:], 0.0)
            m_sb = rpool.tile([P, E * CC], F32)   # one-hot per expert
            s_sb = rpool.tile([P, E * CC], F32)   # within-row inclusive scans
            # one-hot compares on GpSimd so they overlap the DVE scan chain
            for e in range(E):
                nc.gpsimd.tensor_scalar(
                    m_sb[:, e * CC:(e + 1) * CC], te_sb[:], e, None,
                    op0=AluOp.is_equal)
            for e in range(E):
                nc.vector.tensor_tensor_scan(
                    s_sb[:, e * CC:(e + 1) * CC], m_sb[:, e * CC:(e + 1) * CC],
                    zero[:], initial=0.0, op0=AluOp.add, op1=AluOp.add)
            # cross-partition exclusive carry: carry[p,e] = sum_{q<p} rowtotal[q,e]
            r_sb = rpool.tile([P, E], F32)
            s_view = s_sb[:].rearrange("p (e c) -> p e c", e=E)
            nc.vector.tensor_copy(r_sb[:], s_view[:, :, CC - 1])
            # PE LDW has a tight sync-wait budget: matmul operands must come
            # from a single producer engine, so stage DMA'd constants through
            # DVE copies.
            u2_sb = rpool.tile([P, P], F32)
            nc.vector.tensor_copy(u2_sb[:], u_sb[:])
            sel2_sb = rpool.tile([P, NT], F32)
            nc.vector.tensor_copy(sel2_sb[:], sel_sb[:])
            carry_ps = ppool.tile([P, E], F32)
            nc.tensor.matmul(carry_ps[:], u2_sb[:], r_sb[:], start=True, stop=True)
            # d[p,e] = CAP - carry[p,e]; entry valid iff scan <= d
            d_sb = rpool.tile([P, E], F32)
            nc.vector.tensor_scalar(
                d_sb[:], carry_ps[:], -1.0, float(CAP),
                op0=AluOp.mult, op1=AluOp.add)
            vm = rpool.tile([P, CC], F32)
            for e in range(E):
                if e == 0:
                    nc.vector.scalar_tensor_tensor(
                        vm[:], s_sb[:, 0:CC], d_sb[:, 0:1], m_sb[:, 0:CC],
                        op0=AluOp.is_le, op1=AluOp.mult)
                else:
                    t_e = apool.tile([P, CC], F32)
                    nc.vector.scalar_tensor_tensor(
                        t_e[:], s_sb[:, e * CC:(e + 1) * CC], d_sb[:, e:e + 1],
                        m_sb[:, e * CC:(e + 1) * CC],
                        op0=AluOp.is_le, op1=AluOp.mult)
                    nc.vector.tensor_add(vm[:], vm[:], t_e[:])
            nc.vector.tensor_mul(vm[:], vm[:], w_sb[:])
            # coeff[p,u] (token 128p+u) = sum of the token's two entries
            co_sb = rpool.tile([P, P], F32)
            vv = vm[:].rearrange("p (u two) -> p u two", two=2)
            nc.vector.tensor_add(co_sb[:], vv[:, :, 0], vv[:, :, 1])
            # per-core column select: scale[q,j] = coeff[16k+j, q] via one-hot sel
            sc_ps = ppool.tile([P, NT], F32)
            nc.tensor.matmul(sc_ps[:], co_sb[:], sel2_sb[:], start=True, stop=True)
            sc_sb = rpool.tile([P, NT], F32)
            nc.vector.tensor_copy(sc_sb[:], sc_ps[:])

            # ---- main stream: y = coeff * x + bias ----
            # All loads issued first (Sync/SP ring) so a compute-gated store
            # can never sit ahead of a load in the same DGE FIFO; stores go
            # out on the ACT ring.
            xtiles = []
            for j in range(NT):
                t = xpool.tile([P, D], F32)
                nc.sync.dma_start(t[:], xt[j])
                xtiles.append(t)
            for j in range(NT):
                t = xtiles[j]
                nc.vector.scalar_tensor_tensor(
                    t[:], t[:], sc_sb[:, j:j + 1], b_sb[:],
                    op0=AluOp.mult, op1=AluOp.add)
                nc.scalar.dma_start(yt[j], t[:])
    nc.compile()
    return nc


def _get_nc():
    if "nc" not in _CACHE:
        _CACHE["nc"] = _build_bass()
    return _CACHE["nc"]


def kernel(x, cond, mask, scores, expert_weights, top_experts, bias, **run_kwargs):
    x = np.ascontiguousarray(np.asarray(x, dtype=np.float32))
    w = np.ascontiguousarray(np.asarray(expert_weights, dtype=np.float32)).reshape(P, CC)
    te = np.ascontiguousarray(np.asarray(top_experts, dtype=np.int32)).reshape(P, CC)
    bias = np.asarray(bias, dtype=np.float32)
    xf = x.reshape(TOK, D)
    ut = np.triu(np.ones((P, P), np.float32), k=1)
    bbt = np.ascontiguousarray(np.broadcast_to(bias, (P, D)))
    in_maps = []
    for k in range(N_CORES):
        selk = np.zeros((P, NT), np.float32)
        selk[NT * k + np.arange(NT), np.arange(NT)] = 1.0
        in_maps.append({
            "xs": xf[k * TPC:(k + 1) * TPC],
            "te": te, "wf": w, "ut": ut, "sel": selk, "bb": bbt,
        })
    res = run_bass_kernel_spmd(
        _get_nc(), in_maps, core_ids=list(range(N_CORES)), **run_kwargs)
    _CACHE["last_result"] = res
    out = np.concatenate([res.results[k]["ys"] for k in range(N_CORES)], axis=0)
    return out.reshape(B, N, D)


# revision 9
# speedup vs baseline: 1.2049x; 1.2049x over previous
"""MoE routing (capacity-drop dispatch/combine) kernel for 8 Trainium2 cores.

The reference module's expert compute is identity, so binned_gather followed by
binned_scatter algebraically reduces to a per-token scale:

    out[t] = (sum_k expert_weights[t,k] * within_capacity(t,k)) * x[t] + bias

within_capacity(t,k) is determined by the token's position in its expert's bin
under a stable sort of all (token, k) routing entries by expert id, i.e. by the
running per-expert count over the flat entry stream.  The kernel computes that
routing mask on-device with per-expert prefix scans (tensor_tensor_scan along
the free dim + a triangular-matmul carry across partitions), then streams x
through a fused (x * coeff + bias) elementwise pass.

Sharding: data-parallel over tokens; each of the 8 cores scales its own 2048
tokens.  The routing metadata (32K entries) is computed redundantly on every
core, so no collectives are needed.
"""

import numpy as np

import concourse.bass as bass
import concourse.bacc as bacc
import concourse.mybir as mybir
from concourse.tile import TileContext
from concourse.bass_utils import run_bass_kernel_spmd

AluOp = mybir.AluOpType
F32 = mybir.dt.float32
I32 = mybir.dt.int32

N_CORES = 8
B, N, D = 4, 4096, 1024
TOP_K = 2
E = 8
TOK = B * N                # 16384 tokens
T = TOK * TOP_K            # 32768 routing entries
CAP = T // E               # 4096 expert capacity
P = 128                    # partitions
CC = T // P                # 256 routing entries per partition row
TPC = TOK // N_CORES       # 2048 tokens per core
NT = TPC // P              # 16 x-tiles of [128, D] per core

_CACHE = {}


def _build_bass():
    nc = bacc.Bacc(None, target_bir_lowering=False)
    xs = nc.dram_tensor("xs", [TPC, D], F32, kind="ExternalInput")
    te = nc.dram_tensor("te", [P, CC], I32, kind="ExternalInput")
    wf = nc.dram_tensor("wf", [P, CC], F32, kind="ExternalInput")
    ut = nc.dram_tensor("ut", [P, P], F32, kind="ExternalInput")
    sel = nc.dram_tensor("sel", [P, NT], F32, kind="ExternalInput")
    bb = nc.dram_tensor("bb", [P, D], F32, kind="ExternalInput")
    ys = nc.dram_tensor("ys", [TPC, D], F32, kind="ExternalOutput")

    xt = xs.rearrange("(j p) d -> j p d", p=P)
    yt = ys.rearrange("(j p) d -> j p d", p=P)

    with TileContext(nc) as tc:
        with tc.tile_pool(name="const", bufs=1) as cpool, \
             tc.tile_pool(name="route", bufs=1) as rpool, \
             tc.tile_pool(name="acc", bufs=2) as apool, \
             tc.tile_pool(name="ps", bufs=1, space="PSUM") as ppool, \
             tc.tile_pool(name="xw", bufs=NT + 2) as xpool:
            te_sb = cpool.tile([P, CC], I32)
            nc.sync.dma_start(te_sb[:], te[:])
            w_sb = cpool.tile([P, CC], F32)
            nc.sync.dma_start(w_sb[:], wf[:])
            u_sb = cpool.tile([P, P], F32)
            nc.sync.dma_start(u_sb[:], ut[:])
            sel_sb = cpool.tile([P, NT], F32)
            nc.sync.dma_start(sel_sb[:], sel[:])
            b_sb = cpool.tile([P, D], F32)
            nc.sync.dma_start(b_sb[:], bb[:])

            # ---- routing: global capacity mask (redundant on every core) ----
            # Flat entry i = p*CC + c lives at [p, c]; stable-sort bin position
            # equals the global running count of entry's expert over i.
            zero = rpool.tile([P, CC], F32)
            nc.vector.memset(zero[:], 0.0)
            m_sb = rpool.tile([P, E * CC], F32)   # one-hot per expert
            s_sb = rpool.tile([P, E * CC], F32)   # within-row inclusive scans
            for e in range(E):
                nc.vector.tensor_scalar(
                    m_sb[:, e * CC:(e + 1) * CC], te_sb[:], e, None,
                    op0=AluOp.is_equal)
            for e in range(E):
                nc.vector.tensor_tensor_scan(
                    s_sb[:, e * CC:(e + 1) * CC], m_sb[:, e * CC:(e + 1) * CC],
                    zero[:], initial=0.0, op0=AluOp.add, op1=AluOp.add)
            # cross-partition exclusive carry: carry[p,e] = sum_{q<p} rowtotal[q,e]
            r_sb = rpool.tile([P, E], F32)
            s_view = s_sb[:].rearrange("p (e c) -> p e c", e=E)
            nc.vector.tensor_copy(r_sb[:], s_view[:, :, CC - 1])
            # PE LDW has a tight sync-wait budget: matmul operands must come
            # from a single producer engine, so stage DMA'd constants through
            # DVE copies.
            u2_sb = rpool.tile([P, P], F32)
            nc.vector.tensor_copy(u2_sb[:], u_sb[:])
            sel2_sb = rpool.tile([P, NT], F32)
            nc.vector.tensor_copy(sel2_sb[:], sel_sb[:])
            carry_ps = ppool.tile([P, E], F32)
            nc.tensor.matmul(carry_ps[:], u2_sb[:], r_sb[:], start=True, stop=True)
            # d[p,e] = CAP - carry[p,e]; entry valid iff scan <= d
            d_sb = rpool.tile([P, E], F32)
            nc.vector.tensor_scalar(
                d_sb[:], carry_ps[:], -1.0, float(CAP),
                op0=AluOp.mult, op1=AluOp.add)
            vm = rpool.tile([P, CC], F32)
            for e in range(E):
                if e == 0:
                    nc.vector.scalar_tensor_tensor(
                        vm[:], s_sb[:, 0:CC], d_sb[:, 0:1], m_sb[:, 0:CC],
                        op0=AluOp.is_le, op1=AluOp.mult)
                else:
                    t_e = apool.tile([P, CC], F32)
                    nc.vector.scalar_tensor_tensor(
                        t_e[:], s_sb[:, e * CC:(e + 1) * CC], d_sb[:, e:e + 1],
                        m_sb[:, e * CC:(e + 1) * CC],
                        op0=AluOp.is_le, op1=AluOp.mult)
                    nc.vector.tensor_add(vm[:], vm[:], t_e[:])
            nc.vector.tensor_mul(vm[:], vm[:], w_sb[:])
            # coeff[p,u] (token 128p+u) = sum of the token's two entries
            co_sb = rpool.tile([P, P], F32)
            vv = vm[:].rearrange("p (u two) -> p u two", two=2)
            nc.vector.tensor_add(co_sb[:], vv[:, :, 0], vv[:, :, 1])
            # per-core column select: scale[q,j] = coeff[16k+j, q] via one-hot sel
            sc_ps = ppool.tile([P, NT], F32)
            nc.tensor.matmul(sc_ps[:], co_sb[:], sel2_sb[:], start=True, stop=True)
            sc_sb = rpool.tile([P, NT], F32)
            nc.vector.tensor_copy(sc_sb[:], sc_ps[:])

            # ---- main stream: y = coeff * x + bias ----
            # All loads issued first (Sync/SP ring) so a compute-gated store
            # can never sit ahead of a load in the same DGE FIFO; stores go
            # out on the ACT ring.
            xtiles = []
            for j in range(NT):
                t = xpool.tile([P, D], F32)
                nc.sync.dma_start(t[:], xt[j])
                xtiles.append(t)
            for j in range(NT):
                t = xtiles[j]
                nc.vector.scalar_tensor_tensor(
                    t[:], t[:], sc_sb[:, j:j + 1], b_sb[:],
                    op0=AluOp.mult, op1=AluOp.add)
                nc.scalar.dma_start(yt[j], t[:])
    nc.compile()
    return nc


def _get_nc():
    if "nc" not in _CACHE:
        _CACHE["nc"] = _build_bass()
    return _CACHE["nc"]


def kernel(x, cond, mask, scores, expert_weights, top_experts, bias, **run_kwargs):
    x = np.ascontiguousarray(np.asarray(x, dtype=np.float32))
    w = np.ascontiguousarray(np.asarray(expert_weights, dtype=np.float32)).reshape(P, CC)
    te = np.ascontiguousarray(np.asarray(top_experts, dtype=np.int32)).reshape(P, CC)
    bias = np.asarray(bias, dtype=np.float32)
    xf = x.reshape(TOK, D)
    ut = np.triu(np.ones((P, P), np.float32), k=1)
    bbt = np.ascontiguousarray(np.broadcast_to(bias, (P, D)))
    in_maps = []
    for k in range(N_CORES):
        selk = np.zeros((P, NT), np.float32)
        selk[NT * k + np.arange(NT), np.arange(NT)] = 1.0
        in_maps.append({
            "xs": xf[k * TPC:(k + 1) * TPC],
            "te": te, "wf": w, "ut": ut, "sel": selk, "bb": bbt,
        })
    res = run_bass_kernel_spmd(
        _get_nc(), in_maps, core_ids=list(range(N_CORES)), **run_kwargs)
    _CACHE["last_result"] = res
    out = np.concatenate([res.results[k]["ys"] for k in range(N_CORES)], axis=0)
    return out.reshape(B, N, D)


# revision 16
# speedup vs baseline: 1.2467x; 1.0347x over previous
"""MoE routing (capacity-drop dispatch/combine) kernel for 8 Trainium2 cores.

The reference module's expert compute is identity, so binned_gather followed by
binned_scatter algebraically reduces to a per-token scale:

    out[t] = (sum_k expert_weights[t,k] * within_capacity(t,k)) * x[t] + bias

within_capacity(t,k) is determined by the token's position in its expert's bin
under a stable sort of all (token, k) routing entries by expert id, i.e. by the
running per-expert count over the flat entry stream.  The kernel computes that
routing mask on-device with per-expert prefix scans (tensor_tensor_scan along
the free dim + a triangular-matmul carry across partitions), then streams x
through a fused (x * coeff + bias) elementwise pass.

Sharding: data-parallel over tokens; each of the 8 cores scales its own 2048
tokens.  The routing metadata (32K entries) is computed redundantly on every
core, so no collectives are needed.
"""

import numpy as np

import concourse.bass as bass
import concourse.bacc as bacc
import concourse.mybir as mybir
from concourse.tile import TileContext
from concourse.bass_utils import run_bass_kernel_spmd

AluOp = mybir.AluOpType
F32 = mybir.dt.float32
I32 = mybir.dt.int32

N_CORES = 8
B, N, D = 4, 4096, 1024
TOP_K = 2
E = 8
TOK = B * N                # 16384 tokens
T = TOK * TOP_K            # 32768 routing entries
CAP = T // E               # 4096 expert capacity
P = 128                    # partitions
CC = T // P                # 256 routing entries per partition row
TPC = TOK // N_CORES       # 2048 tokens per core
NT = TPC // P              # 16 x-tiles of [128, D] per core
NCH = 4                    # x chunks per core (fewer, bigger DMAs)
TPCH = NT // NCH           # tiles per chunk

_CACHE = {}


def _build_bass():
    nc = bacc.Bacc(None, target_bir_lowering=False)
    xs = nc.dram_tensor("xs", [TPC, D], F32, kind="ExternalInput")
    te = nc.dram_tensor("te", [P, CC], I32, kind="ExternalInput")
    wf = nc.dram_tensor("wf", [P, CC], F32, kind="ExternalInput")
    ut = nc.dram_tensor("ut", [P, P], F32, kind="ExternalInput")
    sel = nc.dram_tensor("sel", [P, NT], F32, kind="ExternalInput")
    bb = nc.dram_tensor("bb", [P, D], F32, kind="ExternalInput")
    ys = nc.dram_tensor("ys", [TPC, D], F32, kind="ExternalOutput")

    # chunk view: token row = ch*TPCH*P + j*P + p
    xt = xs.rearrange("(ch j p) d -> ch p j d", p=P, j=TPCH)
    yt = ys.rearrange("(j p) d -> j p d", p=P)

    with TileContext(nc) as tc:
        with tc.tile_pool(name="const", bufs=1) as cpool, \
             tc.tile_pool(name="route", bufs=1) as rpool, \
             tc.tile_pool(name="ps", bufs=1, space="PSUM") as ppool, \
             tc.tile_pool(name="xw", bufs=NCH) as xpool:
            # constants ride the ACT DGE ring so the x stream owns SP
            te_sb = cpool.tile([P, CC], I32)
            nc.scalar.dma_start(te_sb[:], te[:])
            w_sb = cpool.tile([P, CC], F32)
            nc.scalar.dma_start(w_sb[:], wf[:])
            u_sb = cpool.tile([P, P], F32)
            nc.scalar.dma_start(u_sb[:], ut[:])
            sel_sb = cpool.tile([P, NT], F32)
            nc.scalar.dma_start(sel_sb[:], sel[:])
            b_sb = cpool.tile([P, D], F32)
            nc.scalar.dma_start(b_sb[:], bb[:])

            # x loads: NCH big DMAs on the Sync/SP ring, nothing queued ahead
            xtiles = []
            for ch in range(NCH):
                t = xpool.tile([P, TPCH, D], F32)
                nc.sync.dma_start(t[:], xt[ch])
                xtiles.append(t)

            # ---- routing: global capacity mask (redundant on every core) ----
            # Flat entry i = p*CC + c lives at [p, c]; stable-sort bin position
            # equals the global running count of entry's expert over i.
            zero = rpool.tile([P, CC], F32)
            nc.vector.memset(zero[:], 0.0)
            m_sb = rpool.tile([P, E * CC], F32)   # one-hot per expert
            s_sb = rpool.tile([P, E * CC], F32)   # within-row inclusive scans
            for e in range(E):
                nc.vector.tensor_scalar(
                    m_sb[:, e * CC:(e + 1) * CC], te_sb[:], e, None,
                    op0=AluOp.is_equal)
                nc.vector.tensor_tensor_scan(
                    s_sb[:, e * CC:(e + 1) * CC], m_sb[:, e * CC:(e + 1) * CC],
                    zero[:], initial=0.0, op0=AluOp.add, op1=AluOp.add)
            # PE LDW has a tight sync-wait budget: matmul operands must come
            # from a single producer engine, so stage DMA'd constants through
            # DVE copies.
            u2_sb = rpool.tile([P, P], F32)
            nc.vector.tensor_copy(u2_sb[:], u_sb[:])
            sel2_sb = rpool.tile([P, NT], F32)
            nc.vector.tensor_copy(sel2_sb[:], sel_sb[:])
            # cross-partition exclusive carry: carry[p,e] = sum_{q<p} rowtot[q,e]
            s_view = s_sb[:].rearrange("p (e c) -> p e c", e=E)
            r_sb = rpool.tile([P, E], F32)
            nc.vector.tensor_copy(r_sb[:], s_view[:, :, CC - 1])
            carry_ps = ppool.tile([P, E], F32)
            nc.tensor.matmul(carry_ps[:], u2_sb[:], r_sb[:], start=True, stop=True)
            # d[p,e] = CAP - carry[p,e]; entry valid iff scan <= d
            d_sb = rpool.tile([P, E], F32)
            nc.vector.tensor_scalar(
                d_sb[:], carry_ps[:], -1.0, float(CAP),
                op0=AluOp.mult, op1=AluOp.add)
            # valid_e = (S_e <= CAP - carry_e) * M_e, written back over m_sb
            for e in range(E):
                nc.vector.scalar_tensor_tensor(
                    m_sb[:, e * CC:(e + 1) * CC], s_sb[:, e * CC:(e + 1) * CC],
                    d_sb[:, e:e + 1], m_sb[:, e * CC:(e + 1) * CC],
                    op0=AluOp.is_le, op1=AluOp.mult)
            # collapse experts with a 3-level tree of wide adds -> vm [P, CC]
            h = E * CC // 2
            nc.vector.tensor_add(m_sb[:, 0:h], m_sb[:, 0:h], m_sb[:, h:2 * h])
            nc.vector.tensor_add(m_sb[:, 0:h // 2], m_sb[:, 0:h // 2],
                                 m_sb[:, h // 2:h])
            vm = rpool.tile([P, CC], F32)
            nc.vector.tensor_add(vm[:], m_sb[:, 0:CC], m_sb[:, CC:2 * CC])
            nc.vector.tensor_mul(vm[:], vm[:], w_sb[:])
            # coeff[p,u] (token 128p+u) = sum of the token's two entries
            co_sb = rpool.tile([P, P], F32)
            vv = vm[:].rearrange("p (u two) -> p u two", two=2)
            nc.vector.tensor_add(co_sb[:], vv[:, :, 0], vv[:, :, 1])
            # per-core column select: scale[q,j] = coeff[16k+j, q] via one-hot sel
            sc_ps = ppool.tile([P, NT], F32)
            nc.tensor.matmul(sc_ps[:], co_sb[:], sel2_sb[:], start=True, stop=True)
            sc_sb = rpool.tile([P, NT], F32)
            nc.vector.tensor_copy(sc_sb[:], sc_ps[:])

            # ---- main stream: y = coeff * x + bias, in place, stores on ACT ring
            for j in range(NT):
                t = xtiles[j // TPCH]
                sl = t[:, j % TPCH, :]
                nc.vector.scalar_tensor_tensor(
                    sl, sl, sc_sb[:, j:j + 1], b_sb[:],
                    op0=AluOp.mult, op1=AluOp.add)
                nc.scalar.dma_start(yt[j], sl)
    nc.compile()
    return nc


def _get_nc():
    if "nc" not in _CACHE:
        _CACHE["nc"] = _build_bass()
    return _CACHE["nc"]


def kernel(x, cond, mask, scores, expert_weights, top_experts, bias, **run_kwargs):
    x = np.ascontiguousarray(np.asarray(x, dtype=np.float32))
    w = np.ascontiguousarray(np.asarray(expert_weights, dtype=np.float32)).reshape(P, CC)
    te = np.ascontiguousarray(np.asarray(top_experts, dtype=np.int32)).reshape(P, CC)
    bias = np.asarray(bias, dtype=np.float32)
    xf = x.reshape(TOK, D)
    ut = np.triu(np.ones((P, P), np.float32), k=1)
    bbt = np.ascontiguousarray(np.broadcast_to(bias, (P, D)))
    in_maps = []
    for k in range(N_CORES):
        selk = np.zeros((P, NT), np.float32)
        selk[NT * k + np.arange(NT), np.arange(NT)] = 1.0
        in_maps.append({
            "xs": xf[k * TPC:(k + 1) * TPC],
            "te": te, "wf": w, "ut": ut, "sel": selk, "bb": bbt,
        })
    res = run_bass_kernel_spmd(
        _get_nc(), in_maps, core_ids=list(range(N_CORES)), **run_kwargs)
    _CACHE["last_result"] = res
    out = np.concatenate([res.results[k]["ys"] for k in range(N_CORES)], axis=0)
    return out.reshape(B, N, D)


# revision 19
# speedup vs baseline: 1.4297x; 1.1468x over previous
"""MoE routing (capacity-drop dispatch/combine) kernel for 8 Trainium2 cores.

The reference module's expert compute is identity, so binned_gather followed by
binned_scatter algebraically reduces to a per-token scale:

    out[t] = (sum_k expert_weights[t,k] * within_capacity(t,k)) * x[t] + bias

within_capacity(t,k) is determined by the token's position in its expert's bin
under a stable sort of all (token, k) routing entries by expert id, i.e. by the
running per-expert count over the flat entry stream.  The kernel computes that
routing mask on-device with per-expert prefix scans (tensor_tensor_scan along
the free dim + a triangular-matmul carry across partitions), then streams x
through a fused (x * coeff + bias) elementwise pass.

Sharding: data-parallel over tokens; each of the 8 cores scales its own 2048
tokens.  The routing metadata (32K entries) is computed redundantly on every
core, so no collectives are needed.
"""

import numpy as np

import concourse.bass as bass
import concourse.bacc as bacc
import concourse.mybir as mybir
from concourse.tile import TileContext
from concourse.bass_utils import run_bass_kernel_spmd

AluOp = mybir.AluOpType
F32 = mybir.dt.float32
I32 = mybir.dt.int32

N_CORES = 8
B, N, D = 4, 4096, 1024
TOP_K = 2
E = 8
TOK = B * N                # 16384 tokens
T = TOK * TOP_K            # 32768 routing entries
CAP = T // E               # 4096 expert capacity
P = 128                    # partitions
CC = T // P                # 256 routing entries per partition row
TPC = TOK // N_CORES       # 2048 tokens per core
NT = TPC // P              # 16 x-tiles of [128, D] per core
NCH = 4                    # x chunks per core (fewer, bigger DMAs)
TPCH = NT // NCH           # tiles per chunk

_CACHE = {}


def _build_bass():
    nc = bacc.Bacc(None, target_bir_lowering=False)
    xs = nc.dram_tensor("xs", [TPC, D], F32, kind="ExternalInput")
    te = nc.dram_tensor("te", [P, CC], I32, kind="ExternalInput")
    wf = nc.dram_tensor("wf", [P, CC], F32, kind="ExternalInput")
    ut = nc.dram_tensor("ut", [P, P], F32, kind="ExternalInput")
    sel = nc.dram_tensor("sel", [P, NT], F32, kind="ExternalInput")
    bv = nc.dram_tensor("bv", [1, D], F32, kind="ExternalInput")
    ys = nc.dram_tensor("ys", [TPC, D], F32, kind="ExternalOutput")

    # chunk view: token row = ch*TPCH*P + j*P + p
    xt = xs.rearrange("(ch j p) d -> ch p j d", p=P, j=TPCH)
    yt = ys.rearrange("(j p) d -> j p d", p=P)

    with TileContext(nc) as tc:
        with tc.tile_pool(name="const", bufs=1) as cpool, \
             tc.tile_pool(name="route", bufs=1) as rpool, \
             tc.tile_pool(name="ps", bufs=1, space="PSUM") as ppool, \
             tc.tile_pool(name="xw", bufs=NCH) as xpool:
            # te/wf gate the routing critical path: put them FIRST on the
            # Sync/SP ring so they land before the big x chunks saturate HBM.
            te_sb = cpool.tile([P, CC], I32)
            nc.sync.dma_start(te_sb[:], te[:])
            w_sb = cpool.tile([P, CC], F32)
            nc.sync.dma_start(w_sb[:], wf[:])
            # off-critical-path constants ride the ACT ring
            u_sb = cpool.tile([P, P], F32)
            nc.scalar.dma_start(u_sb[:], ut[:])
            sel_sb = cpool.tile([P, NT], F32)
            nc.scalar.dma_start(sel_sb[:], sel[:])
            bias1 = cpool.tile([1, D], F32)
            nc.scalar.dma_start(bias1[:], bv[:])

            # x loads: NCH big DMAs on the Sync/SP ring
            xtiles = []
            for ch in range(NCH):
                t = xpool.tile([P, TPCH, D], F32)
                nc.sync.dma_start(t[:], xt[ch])
                xtiles.append(t)

            # broadcast bias across partitions with a K=1 PE outer product
            # (saves half a MB of HBM traffic vs DMAing a replicated tile)
            ones_sb = rpool.tile([1, P], F32)
            nc.vector.memset(ones_sb[:], 1.0)
            bias2 = rpool.tile([1, D], F32)
            nc.vector.tensor_copy(bias2[:], bias1[:])
            b_ps = ppool.tile([P, D], F32)
            nc.tensor.matmul(b_ps[:, 0:D // 2], ones_sb[:], bias2[:, 0:D // 2],
                             start=True, stop=True)
            nc.tensor.matmul(b_ps[:, D // 2:D], ones_sb[:], bias2[:, D // 2:D],
                             start=True, stop=True)
            b_sb = rpool.tile([P, D], F32)
            nc.scalar.activation(b_sb[:], b_ps[:],
                                 mybir.ActivationFunctionType.Copy)

            # ---- routing: global capacity mask (redundant on every core) ----
            # Flat entry i = p*CC + c lives at [p, c]; stable-sort bin position
            # equals the global running count of entry's expert over i.
            zero = rpool.tile([P, CC], F32)
            nc.vector.memset(zero[:], 0.0)
            m_sb = rpool.tile([P, E * CC], F32)   # one-hot per expert
            s_sb = rpool.tile([P, E * CC], F32)   # within-row inclusive scans
            for e in range(E):
                nc.vector.tensor_scalar(
                    m_sb[:, e * CC:(e + 1) * CC], te_sb[:], e, None,
                    op0=AluOp.is_equal)
                nc.vector.tensor_tensor_scan(
                    s_sb[:, e * CC:(e + 1) * CC], m_sb[:, e * CC:(e + 1) * CC],
                    zero[:], initial=0.0, op0=AluOp.add, op1=AluOp.add)
            # PE LDW has a tight sync-wait budget: matmul operands must come
            # from a single producer engine, so stage DMA'd constants through
            # DVE copies.
            u2_sb = rpool.tile([P, P], F32)
            nc.vector.tensor_copy(u2_sb[:], u_sb[:])
            sel2_sb = rpool.tile([P, NT], F32)
            nc.vector.tensor_copy(sel2_sb[:], sel_sb[:])
            # cross-partition exclusive carry: carry[p,e] = sum_{q<p} rowtot[q,e]
            s_view = s_sb[:].rearrange("p (e c) -> p e c", e=E)
            r_sb = rpool.tile([P, E], F32)
            nc.vector.tensor_copy(r_sb[:], s_view[:, :, CC - 1])
            carry_ps = ppool.tile([P, E], F32)
            nc.tensor.matmul(carry_ps[:], u2_sb[:], r_sb[:], start=True, stop=True)
            # d[p,e] = CAP - carry[p,e]; entry valid iff scan <= d
            d_sb = rpool.tile([P, E], F32)
            nc.vector.tensor_scalar(
                d_sb[:], carry_ps[:], -1.0, float(CAP),
                op0=AluOp.mult, op1=AluOp.add)
            # valid_e = (S_e <= CAP - carry_e) * M_e, written back over m_sb
            for e in range(E):
                nc.vector.scalar_tensor_tensor(
                    m_sb[:, e * CC:(e + 1) * CC], s_sb[:, e * CC:(e + 1) * CC],
                    d_sb[:, e:e + 1], m_sb[:, e * CC:(e + 1) * CC],
                    op0=AluOp.is_le, op1=AluOp.mult)
            # collapse experts with a 3-level tree of wide adds -> vm [P, CC]
            h = E * CC // 2
            nc.vector.tensor_add(m_sb[:, 0:h], m_sb[:, 0:h], m_sb[:, h:2 * h])
            nc.vector.tensor_add(m_sb[:, 0:h // 2], m_sb[:, 0:h // 2],
                                 m_sb[:, h // 2:h])
            vm = rpool.tile([P, CC], F32)
            nc.vector.tensor_add(vm[:], m_sb[:, 0:CC], m_sb[:, CC:2 * CC])
            nc.vector.tensor_mul(vm[:], vm[:], w_sb[:])
            # coeff[p,u] (token 128p+u) = sum of the token's two entries
            co_sb = rpool.tile([P, P], F32)
            vv = vm[:].rearrange("p (u two) -> p u two", two=2)
            nc.vector.tensor_add(co_sb[:], vv[:, :, 0], vv[:, :, 1])
            # per-core column select: scale[q,j] = coeff[16k+j, q] via one-hot sel
            sc_ps = ppool.tile([P, NT], F32)
            nc.tensor.matmul(sc_ps[:], co_sb[:], sel2_sb[:], start=True, stop=True)
            sc_sb = rpool.tile([P, NT], F32)
            nc.vector.tensor_copy(sc_sb[:], sc_ps[:])

            # ---- main stream: y = coeff * x + bias, in place, stores on ACT ring
            for j in range(NT):
                t = xtiles[j // TPCH]
                sl = t[:, j % TPCH, :]
                nc.vector.scalar_tensor_tensor(
                    sl, sl, sc_sb[:, j:j + 1], b_sb[:],
                    op0=AluOp.mult, op1=AluOp.add)
                nc.scalar.dma_start(yt[j], sl)
    nc.compile()
    return nc


def _get_nc():
    if "nc" not in _CACHE:
        _CACHE["nc"] = _build_bass()
    return _CACHE["nc"]


def kernel(x, cond, mask, scores, expert_weights, top_experts, bias, **run_kwargs):
    x = np.ascontiguousarray(np.asarray(x, dtype=np.float32))
    w = np.ascontiguousarray(np.asarray(expert_weights, dtype=np.float32)).reshape(P, CC)
    te = np.ascontiguousarray(np.asarray(top_experts, dtype=np.int32)).reshape(P, CC)
    bias = np.asarray(bias, dtype=np.float32)
    xf = x.reshape(TOK, D)
    ut = np.triu(np.ones((P, P), np.float32), k=1)
    bvt = np.ascontiguousarray(bias.reshape(1, D))
    in_maps = []
    for k in range(N_CORES):
        selk = np.zeros((P, NT), np.float32)
        selk[NT * k + np.arange(NT), np.arange(NT)] = 1.0
        in_maps.append({
            "xs": xf[k * TPC:(k + 1) * TPC],
            "te": te, "wf": w, "ut": ut, "sel": selk, "bv": bvt,
        })
    res = run_bass_kernel_spmd(
        _get_nc(), in_maps, core_ids=list(range(N_CORES)), **run_kwargs)
    _CACHE["last_result"] = res
    out = np.concatenate([res.results[k]["ys"] for k in range(N_CORES)], axis=0)
    return out.reshape(B, N, D)
